# revision 1
# baseline (speedup 1.0000x reference)
"""Negative-sampling loss kernel for Trainium2 (8 NeuronCores, SPMD).

v8: fp8 quad-packed t-side + merged o-strip gathers + t-affine
core assignment.

Each (t, other) pair needs two gathered embedding rows; the kernel is
bound by SWDGE (Q7) work = ~1.8ns/descriptor + per-instruction
overhead.  Items are assigned to cores by t % 8 so all occurrences of
a target word land on one core; within each (t_win, o_win) bucket
pairs are sorted by t and the resulting long equal-t runs pack up to
FOUR pairs into one 512B fp8 descriptor reading a host-built table
QUAD[r] = in_emb[r] x4 (scaled x256 into fp8 normal range).
Descriptors are fill-sorted so half-h pairs occupy a prefix.
The four per-half o-lists are concatenated (each 128-aligned) into one
merged strip gathered by up to four full 1024-idx instructions of 256B
fp8 elems from OUT2[r] = out_emb[r] x2 (scaled x64); tails use exact
num_idxs.  Scores are descaled on the host (/16384) before
softplus + mean in float64.
"""

from contextlib import ExitStack

import numpy as np
import ml_dtypes

import concourse.bacc as bacc
import concourse.mybir as mybir
import concourse.tile as tile
from concourse.bass_utils import run_bass_kernel_spmd

VOCAB = 100000
D = 128
B = 262144
NEG = 5
NCORES = 8
BPC = B // NCORES
WIN = 32768
NWIN = (VOCAB + WIN - 1) // WIN   # 4
CHUNK = 1024               # t-descriptors per super-tile / idxs per gather
P = 4                      # pairs packed per t-descriptor
NQ = 4
STRIP = 8                  # super-tiles per idx strip DMA
W = CHUNK // 16            # 64 idx words per 1024 idxs per partition
OW = P * W                 # words reserved for the merged o-list per tile

_cache = {}


def _build_nc(tile_meta, tile_meta_nh, sc_width):
    """tile_meta[i] = (t_win, o_win, n_t, L, starts, gqs, col0)."""
    n_tiles = len(tile_meta)
    n_strips = (n_tiles + STRIP - 1) // STRIP
    nc = bacc.Bacc(
        "TRN2",
        target_bir_lowering=False,
        debug=False,
        enable_asserts=False,
        num_swdge_queues=NQ,
    )
    quad_in = nc.dram_tensor("quad_in", [VOCAB, P * D], mybir.dt.float8e4, kind="ExternalInput")
    out2 = nc.dram_tensor("out2", [VOCAB, 2 * D], mybir.dt.float8e4, kind="ExternalInput")
    idx = nc.dram_tensor("idx", [n_strips, 128, STRIP * (W + OW)],
                         mybir.dt.int16, kind="ExternalInput")
    sc_out = nc.dram_tensor("sc_out", [128, sc_width], mybir.dt.float32,
                            kind="ExternalOutput")

    qctr = 0
    with tile.TileContext(nc) as tc, ExitStack() as ctx:
        idxp = ctx.enter_context(tc.tile_pool(name="idx", bufs=3))
        gatp = ctx.enter_context(tc.tile_pool(name="gat", bufs=9))
        scrp = ctx.enter_context(tc.tile_pool(name="scr", bufs=6))
        scp = ctx.enter_context(tc.tile_pool(name="sc", bufs=1))

        sc_all = scp.tile([128, sc_width], mybir.dt.float32)
        strip_tile = None
        for t in range(n_tiles):
            s, k = divmod(t, STRIP)
            if k == 0:
                strip_tile = idxp.tile([128, STRIP * (W + OW)], mybir.dt.int16, tag="strip")
                nc.sync.dma_start(out=strip_tile[:], in_=idx[s])
            t_win, o_win, n_t, L, starts, gqs, col0, dirs = tile_meta[t]
            base = (W + OW) * k
            gq_t = -(-n_t // 128)
            w_t = -(-n_t // 16)
            ti = strip_tile[:, base:base + w_t]

            tt = gatp.tile([128, 8 * P * D], mybir.dt.float8e4, tag="tt")
            nc.gpsimd.dma_gather(
                tt[:].rearrange("p (g d) -> p g d", d=P * D)[:, :gq_t, :],
                quad_in[t_win * WIN:, :],
                ti, n_t, n_t, P * D, elem_step=P * D,
                queue_num=qctr % NQ,
            )
            qctr += 1
            # o strip: one exact-count gather per half into its
            # 128-aligned segment; pad slots hold stale data (masked)
            GQL = -(-L // 128)
            ost = gatp.tile([128, P * 8 * 2 * D], mybir.dt.float8e4, tag="os")
            ost3 = ost[:].rearrange("p (g e) -> p g e", e=2 * D)
            ost4 = ost[:].rearrange("p (g c d) -> p g c d", c=2, d=D)
            for st, n in dirs:
                gq = -(-n // 128)
                sh = st // 128
                oi = strip_tile[:, base + W + st // 16:
                                base + W + st // 16 + (-(-n // 16))]
                nc.gpsimd.dma_gather(
                    ost3[:, sh:sh + gq, :],
                    out2[o_win * WIN:, :],
                    oi, n, n, 2 * D, elem_step=2 * D,
                    queue_num=qctr % NQ,
                )
                qctr += 1
            tt4 = tt[:].rearrange("p (g h d) -> p g h d", h=P, d=D)
            oh3 = ost4[:, :, 0, :]
            scr = scrp.tile([128, P * 8 * D], mybir.dt.bfloat16, tag="scr")
            scr3 = scr[:].rearrange("p (g d) -> p g d", d=D)
            for h in range(P):
                gq = gqs[h]
                if gq == 0:
                    continue
                sh = starts[h] // 128
                nc.vector.tensor_tensor(out=scr3[:, sh:sh + gq, :],
                                        in0=oh3[:, sh:sh + gq, :],
                                        in1=tt4[:, :gq, h, :],
                                        op=mybir.AluOpType.mult)
            nc.vector.tensor_reduce(
                out=sc_all[:, col0:col0 + GQL],
                in_=scr3[:, :GQL, :], axis=mybir.AxisListType.X,
                op=mybir.AluOpType.add)
        nc.sync.dma_start(out=sc_out[:], in_=sc_all[:])
    nc.finalize()
    return nc


def _plan_and_pack(target_words, context_words, negative_words):
    tf = np.asarray(target_words).astype(np.int64).ravel()
    cf = np.asarray(context_words).astype(np.int64).ravel()
    nf = np.asarray(negative_words).astype(np.int64).reshape(B, NEG)

    # same-t items co-locate: core = t % NCORES lengthens same-t runs
    core_of = tf % NCORES

    NG = NWIN * NWIN

    per_cg = [[None] * NG for _ in range(NCORES)]
    n_desc = np.zeros((NCORES, NG), np.int64)
    n_pref = np.zeros((NCORES, NG, P), np.int64)
    for c in range(NCORES):
        items = np.nonzero(core_of == c)[0]
        t_c = np.repeat(tf[items], 1 + NEG)
        o_c = np.concatenate([cf[items, None], nf[items]], axis=1).ravel()
        p_c = np.zeros(len(items) * (1 + NEG), bool)
        p_c.reshape(-1, 1 + NEG)[:, 0] = True
        key_c = (t_c // WIN) * NWIN + (o_c // WIN)
        order = np.lexsort((t_c, key_c))
        ks = key_c[order]
        tw = t_c[order]
        ow = o_c[order]
        ps = p_c[order]
        bounds = np.searchsorted(ks, np.arange(NG + 1))
        for g in range(NG):
            lo, hi = bounds[g], bounds[g + 1]
            tv = tw[lo:hi]
            n = hi - lo
            if n == 0:
                per_cg[c][g] = np.zeros((0, 1 + 3 * P), np.int64)
                continue
            run_start = np.concatenate([[0], np.nonzero(tv[1:] != tv[:-1])[0] + 1])
            run_len = np.diff(np.concatenate([run_start, [n]]))
            base_t = (g // NWIN) * WIN
            base_o = (g % NWIN) * WIN
            fills = []
            rows = []
            for rs, rl in zip(run_start, run_len):
                off = 0
                while off < rl:
                    fill = min(P, rl - off)
                    row = np.zeros(1 + 3 * P, np.int64)
                    row[0] = tv[rs] - base_t
                    for h in range(fill):
                        row[1 + h] = ow[lo + rs + off + h] - base_o
                        row[1 + P + h] = 1
                        row[1 + 2 * P + h] = ps[lo + rs + off + h]
                    fills.append(fill)
                    rows.append(row)
                    off += fill
            fills = np.array(fills)
            arr = np.stack(rows)
            order_f = np.argsort(-fills, kind="stable")
            arr = arr[order_f]
            fills = fills[order_f]
            per_cg[c][g] = arr
            n_desc[c, g] = len(arr)
            for h in range(P):
                n_pref[c, g, h] = int((fills > h).sum())

    # shared schedule
    D_g = n_desc.max(axis=0)
    tile_meta = []
    tile_meta_nh = []
    tile_group = []
    col0 = 0
    for g in range(NG):
        n = int(D_g[g])
        off = 0
        while off < n:
            nt = min(CHUNK, n - off)
            n_h = []
            for h in range(P):
                v = int(np.maximum(0, np.minimum(n_pref[:, g, h] - off, nt)).max())
                n_h.append(v)
            # 128-aligned merged o-list segments
            starts = []
            gqs = []
            L = 0
            for h in range(P):
                starts.append(L)
                gq = -(-n_h[h] // 128)
                gqs.append(gq)
                L += gq * 128
            if L == 0:
                starts = [0] * P
                gqs = [1] + [0] * (P - 1)
                L = 128
            dirs = []
            h = 0
            while h < P:
                if n_h[h] == 0:
                    h += 1
                    continue
                st = starts[h]
                end = st + n_h[h]
                j = h + 1
                while j < P and n_h[j] > 0 and (starts[j] + n_h[j] - st) <= CHUNK:
                    end = starts[j] + n_h[j]
                    j += 1
                dirs.append((st, end - st))
                h = j
            tile_meta.append((g // NWIN, g % NWIN, nt, L,
                              tuple(starts), tuple(gqs), col0, tuple(dirs)))
            tile_meta_nh.append(tuple(n_h))
            tile_group.append((g, off, n_h))
            col0 += L // 128
            off += CHUNK
    n_tiles = len(tile_meta)
    sc_width = col0

    tidx = np.zeros((NCORES, n_tiles, CHUNK), np.int16)
    oidx = np.zeros((NCORES, n_tiles, P * CHUNK), np.int16)
    vH = np.zeros((NCORES, n_tiles, P, CHUNK), bool)
    pH = np.zeros((NCORES, n_tiles, P, CHUNK), bool)
    for c in range(NCORES):
        for ti_, ((g, off, n_h), meta) in enumerate(zip(tile_group, tile_meta)):
            seg = per_cg[c][g][off:off + CHUNK]
            m = len(seg)
            if m == 0:
                continue
            starts = meta[4]
            tidx[c, ti_, :m] = seg[:, 0]
            for h in range(P):
                mh = min(m, n_h[h])
                # per-core descs with fill>h are a prefix of length own_pref,
                # but indices are defined (0) beyond it; mask handles validity
                oidx[c, ti_, starts[h]:starts[h] + mh] = seg[:mh, 1 + h]
                vH[c, ti_, h, :mh] = seg[:mh, 1 + P + h].astype(bool)
                pH[c, ti_, h, :mh] = seg[:mh, 1 + 2 * P + h].astype(bool)

    n_strips = (n_tiles + STRIP - 1) // STRIP
    n_pad = n_strips * STRIP

    def wrap16(v, words):  # [n, words*16] -> [n, 128, words]
        w_ = v.reshape(v.shape[0], words, 16).transpose(0, 2, 1)
        return np.tile(w_, (1, 8, 1))

    per_core = []
    for c in range(NCORES):
        allw = np.zeros((n_pad, 128, W + OW), np.int16)
        allw[:n_tiles, :, :W] = wrap16(tidx[c], W)
        allw[:n_tiles, :, W:] = wrap16(oidx[c], OW)
        strips = (allw.reshape(n_strips, STRIP, 128, W + OW)
                  .transpose(0, 2, 1, 3)
                  .reshape(n_strips, 128, STRIP * (W + OW)).copy())
        per_core.append(strips)
    return tile_meta, tile_meta_nh, per_core, (vH, pH), n_tiles, sc_width


def _unpack(sc_outs, masks, tile_meta, n_tiles):
    vH, pH = masks
    pos_sum = 0.0
    neg_sum = 0.0
    for c in range(len(sc_outs)):
        sc = np.asarray(sc_outs[c]).astype(np.float64) / (256.0 * 64.0)
        for ti_, meta in enumerate(tile_meta):
            starts, gqs, col0 = meta[4], meta[5], meta[6]
            for h in range(len(gqs)):
                gq = gqs[h]
                if gq == 0:
                    continue
                block = sc[:, col0 + starts[h] // 128:col0 + starts[h] // 128 + gq]
                vals = block.T.reshape(-1)          # k = g*128 + p
                v = vH[c, ti_, h, :gq * 128]
                p = pH[c, ti_, h, :gq * 128]
                pos_sum += np.logaddexp(0.0, -vals[v & p]).sum()
                neg_sum += np.logaddexp(0.0, vals[v & ~p]).sum()
    positive_loss = np.float32(pos_sum / B)
    negative_loss = np.float32(neg_sum / (B * NEG))
    return positive_loss, negative_loss


def kernel(target_words, context_words, negative_words, input_emb, output_emb,
           _want_results=False, _trace=False):
    input_emb = np.asarray(input_emb)
    output_emb = np.asarray(output_emb)
    in_f8 = (input_emb * 256.0).astype(ml_dtypes.float8_e4m3)
    out_f8 = (output_emb * 64.0).astype(ml_dtypes.float8_e4m3)
    quad_in = np.tile(in_f8, (1, P)).copy()
    out2 = np.tile(out_f8, (1, 2)).copy()

    tile_meta, tile_meta_nh, per_core, masks, n_tiles, sc_width = _plan_and_pack(
        target_words, context_words, negative_words)

    cache_key = (tuple(tile_meta), tuple(tile_meta_nh))
    if cache_key not in _cache:
        _cache[cache_key] = _build_nc(tile_meta, tile_meta_nh, sc_width)
    nc = _cache[cache_key]

    in_maps = []
    for c in range(NCORES):
        in_maps.append({
            "quad_in": quad_in,
            "out2": out2,
            "idx": per_core[c],
        })
    br = run_bass_kernel_spmd(nc, in_maps, core_ids=list(range(NCORES)),
                              trace=_trace)

    positive_loss, negative_loss = _unpack(
        [br.results[c]["sc_out"] for c in range(NCORES)], masks, tile_meta, n_tiles)
    if _want_results:
        return (positive_loss, negative_loss), br
    return (positive_loss, negative_loss)

